# revision 26
# baseline (speedup 1.0000x reference)
"""Decagon GNN (3-layer multi-relation graph conv) on 8 Trainium2 NeuronCores.

Strategy (dst-sharded graph parallel):
  - Pad node counts: N0 30000->30720 (8 x 3840), N1 6000->6144 (8 x 768).
    Core c owns dst rows [c*shard, (c+1)*shard) of each node type.
  - Per layer, per relation: Z = X @ W computed row-sharded on the owning
    core (PE, bf16), then AllGather -> full Z table in each core's HBM.
  - Sparse A @ Z:
      a00/a01/a10: edges sorted by dst block; per 128-edge chunk, gather the
      128 source rows of Z with dma_gather ([128 edges x 512] SBUF tile),
      build a one-hot scatter matrix S[e,d] = val[e] * (rowoff[e] == d) with
      one DVE tensor_scalar op from a constant iota, and accumulate
      psum[128 dst, 512] += S.T @ G on the PE across all chunks of a block.
      a11a/a11b: dense-densified A^T blocks (host-built, bf16) x Z tiles on
      the PE (density ~1.4 edges per contraction slot beats per-edge work).
  - ReLU epilogue on ACT; e^T persisted (PE transpose) as next layer's lhsT.
  - Outputs [e1, e1, e3] written straight into per-core output slabs.
"""

import math
from contextlib import ExitStack

import ml_dtypes
import numpy as np

import concourse.bacc as bacc
import concourse.bass as bass
import concourse.mybir as mybir
import concourse.tile as tile
from concourse import library_config
from concourse.bass_utils import run_bass_kernel_spmd

P = 128
BF16 = mybir.dt.bfloat16
F32 = mybir.dt.float32
I16 = mybir.dt.int16
NPBF16 = ml_dtypes.bfloat16


def default_cfg():
    return dict(N0=30000, N1=6000, F=1024, H=512, NC=8, GG=8)


def _pad_rows(x, n):
    if x.shape[0] == n:
        return x
    out = np.zeros((n,) + x.shape[1:], dtype=x.dtype)
    out[: x.shape[0]] = x
    return out


def _prep_featT(feat, n_pad, nc_cores):
    """feat [N, F] f32 -> per-core lhsT tile array [B, P(f), KT, P(rows)] bf16."""
    N, F = feat.shape
    shard = n_pad // nc_cores
    B = shard // P
    KT = F // P
    fp = _pad_rows(np.asarray(feat), n_pad).astype(NPBF16)
    out = []
    for c in range(nc_cores):
        sh = fp[c * shard : (c + 1) * shard]  # [shard, F]
        t = sh.reshape(B, P, KT, P).transpose(0, 3, 2, 1)  # [B, f, kt, r]
        # want DRAM [B, p=f, kt, r]: element [rb, f, kt, r] = X[rb*P+r, kt*P+f]
        out.append(np.ascontiguousarray(t))
    return out


def _prep_scatter(row, col, val, n_dst_pad, nc_cores, GG):
    """Edge prep for PE-scatter relations.

    Returns (sched nch[B] shared across cores,
             per-core dicts with idx [P, NCH*8] i16, roff [P, NCH] f32,
             val [P, NCH] f32)."""
    row = np.asarray(row).astype(np.int64)
    col = np.asarray(col).astype(np.int64)
    val = np.asarray(val).astype(np.float32)
    shard = n_dst_pad // nc_cores
    B = shard // P
    percore = []
    cnts = np.zeros((nc_cores, B), dtype=np.int64)
    for c in range(nc_cores):
        m = (row // shard) == c
        r = row[m] - c * shard
        cl = col[m]
        v = val[m]
        o = np.argsort(r // P, kind="stable")
        r, cl, v = r[o], cl[o], v[o]
        blk = r // P
        cnts[c] = np.bincount(blk, minlength=B)
        percore.append((r, cl, v, blk))
    nch = np.maximum(np.ceil(cnts / P).astype(np.int64).max(axis=0), 1)
    # pad total chunk count to a multiple of GG by extending the last block
    tot = int(nch.sum())
    nch[-1] += (-tot) % GG
    tot = int(nch.sum())
    starts = np.concatenate([[0], np.cumsum(nch)])

    data = []
    for c in range(nc_cores):
        r, cl, v, blk = percore[c]
        cols_p = np.zeros(tot * P, dtype=np.int64)
        roff_p = np.zeros(tot * P, dtype=np.float32)
        val_p = np.zeros(tot * P, dtype=np.float32)
        bstart = np.concatenate([[0], np.cumsum(np.bincount(blk, minlength=B))])
        for b in range(B):
            n = bstart[b + 1] - bstart[b]
            pos = starts[b] * P
            cols_p[pos : pos + n] = cl[bstart[b] : bstart[b + 1]]
            roff_p[pos : pos + n] = r[bstart[b] : bstart[b + 1]] % P
            val_p[pos : pos + n] = v[bstart[b] : bstart[b + 1]]
        n16 = tot * P // 16
        idx = cols_p.astype(np.int16).reshape(n16, 16).T  # [16, n16]
        idx = np.ascontiguousarray(np.tile(idx, (8, 1)))  # [128, n16]
        data.append(
            dict(
                idx=idx,
                roff=np.ascontiguousarray(roff_p.reshape(tot, P).T),
                val=np.ascontiguousarray(val_p.reshape(tot, P).T),
            )
        )
    return nch.tolist(), data


def _prep_dense(row, col, val, n_dst_pad, n_src_pad, nc_cores):
    """Dense A^T per core: [KT(src tiles), P(src in tile), shard(dst)] bf16."""
    row = np.asarray(row).astype(np.int64)
    col = np.asarray(col).astype(np.int64)
    val = np.asarray(val).astype(np.float32)
    shard = n_dst_pad // nc_cores
    KT = n_src_pad // P
    out = []
    for c in range(nc_cores):
        m = (row // shard) == c
        A = np.zeros((shard, n_src_pad), dtype=np.float32)
        np.add.at(A, (row[m] - c * shard, col[m]), val[m])
        At = np.ascontiguousarray(A.T.reshape(KT, P, shard)).astype(NPBF16)
        out.append(At)
    return out


def build_program(cfg, nch00, nch01, nch10, reps=1):
    N0P = cfg["N0P"]
    N1P = cfg["N1P"]
    F = cfg["F"]
    H = cfg["H"]
    NC = cfg["NC"]
    GG = cfg["GG"]
    S0 = N0P // NC
    S1 = N1P // NC
    B0 = S0 // P
    B1 = S1 // P
    KF = F // P
    KH = H // P
    K1 = N1P // P  # src tiles for dense a11
    RG = [list(range(NC))]

    nc = bacc.Bacc("TRN2")

    # ---------------- I/O declarations ----------------
    feat0T = nc.declare_dram_parameter("feat0T", [B0, P, KF, P], BF16, isOutput=False)
    feat1T = nc.declare_dram_parameter("feat1T", [B1, P, KF, P], BF16, isOutput=False)
    Wd = {}
    for l in (1, 2, 3):
        kt = KF if l == 1 else KH
        for r in ("00", "01", "10", "11a", "11b"):
            Wd[(l, r)] = nc.declare_dram_parameter(
                f"W{l}_{r}", [P, kt, H], BF16, isOutput=False
            )
    edge = {}
    for r, nchr in (("a00", nch00), ("a01", nch01), ("a10", nch10)):
        tot = int(sum(nchr))
        edge[r] = dict(
            idx=nc.declare_dram_parameter(f"{r}_idx", [P, tot * 8], I16, isOutput=False),
            roff=nc.declare_dram_parameter(f"{r}_roff", [P, tot], F32, isOutput=False),
            val=nc.declare_dram_parameter(f"{r}_val", [P, tot], F32, isOutput=False),
            nch=nchr,
            tot=tot,
        )
    A11aT = nc.declare_dram_parameter("A11aT", [K1, P, S1], BF16, isOutput=False)
    A11bT = nc.declare_dram_parameter("A11bT", [K1, P, S1], BF16, isOutput=False)
    iotaC = nc.declare_dram_parameter("iotaC", [P, P], F32, isOutput=False)
    identC = nc.declare_dram_parameter("identC", [P, P], BF16, isOutput=False)
    out0 = nc.declare_dram_parameter("out0", [S0, 3 * H], F32, isOutput=True)
    out1 = nc.declare_dram_parameter("out1", [S1, 3 * H], F32, isOutput=True)

    # ---------------- internal DRAM ----------------
    zsh = {}
    zfull = {}
    for r, npad in (("00", N0P), ("10", N0P), ("01", N1P), ("11a", N1P), ("11b", N1P)):
        zsh[r] = nc.dram_tensor(f"zsh_{r}", [npad // NC, H], BF16)
        zfull[r] = nc.dram_tensor(f"zfull_{r}", [npad, H], BF16, addr_space="Shared")
    # e^T ping-pong buffers, layout [B, P(f), KH, P(rows)]
    e0T = [nc.dram_tensor(f"e0T_{i}", [B0, P, KH, P], BF16) for i in range(2)]
    e1T = [nc.dram_tensor(f"e1T_{i}", [B1, P, KH, P], BF16) for i in range(2)]

    with tile.TileContext(nc) as tc, ExitStack() as ctx:
        pool_const = ctx.enter_context(tc.tile_pool(name="const", bufs=1))
        pool_meta = ctx.enter_context(tc.tile_pool(name="meta", bufs=1))
        pool_w = ctx.enter_context(tc.tile_pool(name="w", bufs=2))
        pool_lhs = ctx.enter_context(tc.tile_pool(name="lhs", bufs=3))
        pool_zb = ctx.enter_context(tc.tile_pool(name="zb", bufs=3))
        pool_g = ctx.enter_context(tc.tile_pool(name="g", bufs=4))
        pool_s = ctx.enter_context(tc.tile_pool(name="s", bufs=4))
        pool_d = ctx.enter_context(tc.tile_pool(name="d", bufs=2))
        pool_a = ctx.enter_context(tc.tile_pool(name="a", bufs=2))
        pool_eb = ctx.enter_context(tc.tile_pool(name="eb", bufs=3))
        pool_trs = ctx.enter_context(tc.tile_pool(name="trs", bufs=3))
        pool_acc = ctx.enter_context(tc.tile_pool(name="acc", bufs=1))
        ps_z = ctx.enter_context(tc.tile_pool(name="ps_z", bufs=2, space="PSUM"))
        ps_s0 = ctx.enter_context(tc.tile_pool(name="ps_s0", bufs=2, space="PSUM"))
        ps_s1 = ctx.enter_context(tc.tile_pool(name="ps_s1", bufs=2, space="PSUM"))
        ps_tr = ctx.enter_context(tc.tile_pool(name="ps_tr", bufs=2, space="PSUM"))

        nc.gpsimd.load_library(library_config.mlp)

        ident = pool_const.tile([P, P], BF16)
        nc.sync.dma_start(ident[:], identC[:, :])
        iota = pool_const.tile([P, P], F32)
        nc.sync.dma_start(iota[:], iotaC[:, :])

        nidx_reg = nc.gpsimd.to_reg(GG * P)

        # resident edge metadata (reused by all 3 layers)
        meta = {}
        for r in ("a00", "a01", "a10"):
            tot = edge[r]["tot"]
            idx_sb = pool_meta.tile([P, tot * 8], I16, name=f"{r}_idx")
            nc.sync.dma_start(idx_sb[:], edge[r]["idx"][:, :])
            roff_sb = pool_meta.tile([P, tot], F32, name=f"{r}_roff")
            nc.sync.dma_start(roff_sb[:], edge[r]["roff"][:, :])
            val_sb = pool_meta.tile([P, tot], F32, name=f"{r}_val")
            nc.sync.dma_start(val_sb[:], edge[r]["val"][:, :])
            meta[r] = (idx_sb, roff_sb, val_sb)

        class GatherStream:
            """Streams G tiles ([P, GG, H] groups) for one scatter relation."""

            def __init__(self, rel, table_ap):
                self.idx_sb = meta[rel][0]
                self.table = table_ap
                self.i = 0
                self.cur = None
                self.rel = rel

            def next_slice(self):
                g, j = divmod(self.i, GG)
                if j == 0:
                    self.cur = pool_g.tile([P, GG, H], BF16, tag="g")
                    nc.gpsimd.dma_gather(
                        out_ap=self.cur[:, :, :],
                        in_ap=self.table[:, :],
                        idxs_ap=self.idx_sb[:, g * GG * 8 : (g + 1) * GG * 8],
                        num_idxs=GG * P,
                        num_idxs_reg=nidx_reg,
                        elem_size=H,
                    )
                self.i += 1
                return self.cur[:, j, :]

        def s_chunk_matmul(rel, ci, psum, start, stop, gs):
            """One 128-edge scatter chunk: build S, psum += S.T @ G."""
            _, roff_sb, val_sb = meta[rel]
            g_slice = gs.next_slice()
            s_tile = pool_s.tile([P, P], BF16, tag="s")
            nc.vector.tensor_scalar(
                out=s_tile[:],
                in0=iota[:],
                scalar1=roff_sb[:, ci : ci + 1],
                scalar2=val_sb[:, ci : ci + 1],
                op0=mybir.AluOpType.is_equal,
                op1=mybir.AluOpType.mult,
            )
            nc.tensor.matmul(
                psum[:], lhsT=s_tile[:], rhs=g_slice, start=start, stop=stop
            )

        def z_phase(l, srcT0, srcT1):
            KT = KF if l == 1 else KH
            for r, srcT, B in (
                ("00", srcT0, B0),
                ("10", srcT0, B0),
                ("01", srcT1, B1),
                ("11a", srcT1, B1),
                ("11b", srcT1, B1),
            ):
                w_sb = pool_w.tile([P, KT, H], BF16, tag="w")
                nc.sync.dma_start(w_sb[:], Wd[(l, r)][:, :, :])
                for rb in range(B):
                    lhs = pool_lhs.tile([P, KT, P], BF16, tag="lhs")
                    nc.sync.dma_start(lhs[:], srcT[rb])
                    ps = ps_z.tile([P, H], F32, tag="psz")
                    for kt in range(KT):
                        nc.tensor.matmul(
                            ps[:],
                            lhsT=lhs[:, kt, :],
                            rhs=w_sb[:, kt, :],
                            start=(kt == 0),
                            stop=(kt == KT - 1),
                        )
                    zb = pool_zb.tile([P, H], BF16, tag="zb")
                    nc.scalar.copy(zb[:], ps[:])
                    nc.sync.dma_start(zsh[r][rb * P : (rb + 1) * P, :], zb[:])
                nc.gpsimd.collective_compute(
                    "AllGather",
                    mybir.AluOpType.bypass,
                    ins=[zsh[r][:, :]],
                    outs=[zfull[r][:, :]],
                    replica_groups=RG,
                )

        def epilogue(l, psum, rb, out_dram, eT_dram, acc=None):
            """ReLU (l<3), write outputs (l in {1,3}), persist e^T (l<3).

            acc: optional SBUF f32 tile added to psum first (type-1 dense part).
            """
            if acc is not None:
                src = pool_eb.tile([P, H], F32, tag="sum32")
                nc.vector.tensor_tensor(
                    out=src[:], in0=psum[:], in1=acc[:], op=mybir.AluOpType.add
                )
            else:
                src = psum
            if l == 1:
                eb32 = pool_eb.tile([P, H], F32, tag="eb32")
                nc.scalar.activation(eb32[:], src[:], mybir.ActivationFunctionType.Relu)
                rows = slice(rb * P, (rb + 1) * P)
                nc.sync.dma_start(out_dram[rows, 0:H], eb32[:])
                nc.sync.dma_start(out_dram[rows, H : 2 * H], eb32[:])
                eb = pool_eb.tile([P, H], BF16, tag="eb")
                nc.vector.tensor_copy(eb[:], eb32[:])
            elif l == 2:
                eb = pool_eb.tile([P, H], BF16, tag="eb")
                nc.scalar.activation(eb[:], src[:], mybir.ActivationFunctionType.Relu)
            else:
                eb32 = pool_eb.tile([P, H], F32, tag="eb32")
                nc.scalar.copy(eb32[:], src[:])
                rows = slice(rb * P, (rb + 1) * P)
                nc.sync.dma_start(out_dram[rows, 2 * H : 3 * H], eb32[:])
                return
            # persist e^T tiles for next layer's lhsT (one PSUM bank, 4 slices)
            stage = pool_trs.tile([P, KH, P], BF16, tag="trs")
            pst = ps_tr.tile([P, H], BF16, tag="pst")
            for fc in range(KH):
                nc.tensor.transpose(
                    pst[:, fc * P : (fc + 1) * P],
                    eb[:, fc * P : (fc + 1) * P],
                    ident[:],
                )
            nc.scalar.copy(stage[:].rearrange("p a b -> p (a b)"), pst[:])
            nc.sync.dma_start(eT_dram[rb], stage[:])

        def s0_phase(l, eT_out):
            gs01 = GatherStream("a01", zfull["01"][:, :])
            gs00 = GatherStream("a00", zfull["00"][:, :])
            nch01 = edge["a01"]["nch"]
            nch00 = edge["a00"]["nch"]
            c01 = 0
            c00 = 0
            for b in range(B0):
                ps = ps_s0.tile([P, H], F32, tag="ps0")
                n1 = nch01[b]
                n0 = nch00[b]
                for i in range(n1):
                    s_chunk_matmul("a01", c01 + i, ps, start=(i == 0), stop=False, gs=gs01)
                for i in range(n0):
                    s_chunk_matmul(
                        "a00", c00 + i, ps, start=False, stop=(i == n0 - 1), gs=gs00
                    )
                c01 += n1
                c00 += n0
                epilogue(l, ps, b, out0, eT_out)

        def s1_phase(l, eT_out):
            # dense a11a/a11b: k-grouped PE accumulation, spilled to SBUF f32
            # accumulators per dst block (keeps PSUM usage at 2 banks).
            KGRP = 4
            accs = [
                pool_acc.tile([P, H], F32, tag=f"acc{b}", name=f"acc{b}")
                for b in range(B1)
            ]
            for g in range(0, K1, KGRP):
                kts = range(g, min(g + KGRP, K1))
                rh = {}
                for kt in kts:
                    rha = pool_d.tile([P, H], BF16, tag=f"rha{kt % KGRP}")
                    nc.sync.dma_start(rha[:], zfull["11a"][kt * P : (kt + 1) * P, :])
                    rhb = pool_d.tile([P, H], BF16, tag=f"rhb{kt % KGRP}")
                    nc.sync.dma_start(rhb[:], zfull["11b"][kt * P : (kt + 1) * P, :])
                    asb = pool_a.tile([P, S1], BF16, tag=f"asb{kt % KGRP}")
                    nc.sync.dma_start(asb[:], A11aT[kt])
                    bsb = pool_a.tile([P, S1], BF16, tag=f"bsb{kt % KGRP}")
                    nc.sync.dma_start(bsb[:], A11bT[kt])
                    rh[kt] = (rha, rhb, asb, bsb)
                for b in range(B1):
                    ds = slice(b * P, (b + 1) * P)
                    ps = ps_s1.tile([P, H], F32, tag="ps1")
                    for i, kt in enumerate(kts):
                        rha, rhb, asb, bsb = rh[kt]
                        nc.tensor.matmul(
                            ps[:], lhsT=asb[:, ds], rhs=rha[:],
                            start=(i == 0), stop=False,
                        )
                        nc.tensor.matmul(
                            ps[:], lhsT=bsb[:, ds], rhs=rhb[:],
                            start=False, stop=(i == len(kts) - 1),
                        )
                    if g == 0:
                        nc.vector.tensor_copy(accs[b][:], ps[:])
                    else:
                        nc.vector.tensor_tensor(
                            out=accs[b][:], in0=accs[b][:], in1=ps[:],
                            op=mybir.AluOpType.add,
                        )
            gs10 = GatherStream("a10", zfull["10"][:, :])
            nch10 = edge["a10"]["nch"]
            c10 = 0
            for b in range(B1):
                n = nch10[b]
                ps = ps_s1.tile([P, H], F32, tag="ps1")
                for i in range(n):
                    s_chunk_matmul(
                        "a10", c10 + i, ps, start=(i == 0), stop=(i == n - 1), gs=gs10
                    )
                c10 += n
                epilogue(l, ps, b, out1, eT_out, acc=accs[b])

        for _rep in range(reps):
            for l in (1, 2, 3):
                if l == 1:
                    srcT0, srcT1 = feat0T, feat1T
                elif l == 2:
                    srcT0, srcT1 = e0T[0], e1T[0]
                else:
                    srcT0, srcT1 = e0T[1], e1T[1]
                eT0_out = e0T[0] if l == 1 else e0T[1]
                eT1_out = e1T[0] if l == 1 else e1T[1]
                z_phase(l, srcT0, srcT1)
                s0_phase(l, eT0_out)
                s1_phase(l, eT1_out)

    nc.compile()
    return nc


def prepare(cfg, inputs):
    """Host-side sharding/preprocessing. Returns (nc, in_maps)."""
    NC = cfg["NC"]
    GG = cfg["GG"]
    cfg["N0P"] = N0P = -(-cfg["N0"] // (NC * P)) * (NC * P)
    cfg["N1P"] = N1P = -(-cfg["N1"] // (NC * P)) * (NC * P)
    H = cfg["H"]
    F = cfg["F"]

    f0 = _prep_featT(np.asarray(inputs["feat0"], np.float32), N0P, NC)
    f1 = _prep_featT(np.asarray(inputs["feat1"], np.float32), N1P, NC)
    nch00, d00 = _prep_scatter(
        inputs["a00_row"], inputs["a00_col"], inputs["a00_val"], N0P, NC, GG
    )
    nch01, d01 = _prep_scatter(
        inputs["a01_row"], inputs["a01_col"], inputs["a01_val"], N0P, NC, GG
    )
    nch10, d10 = _prep_scatter(
        inputs["a10_row"], inputs["a10_col"], inputs["a10_val"], N1P, NC, GG
    )
    da = _prep_dense(
        inputs["a11a_row"], inputs["a11a_col"], inputs["a11a_val"], N1P, N1P, NC
    )
    db = _prep_dense(
        inputs["a11b_row"], inputs["a11b_col"], inputs["a11b_val"], N1P, N1P, NC
    )

    KF = F // P
    KH = H // P
    wmaps = {}
    for l in (1, 2, 3):
        kt = KF if l == 1 else KH
        for r in ("00", "01", "10", "11a", "11b"):
            w = np.asarray(inputs[f"W{l}_{r}"], np.float32)
            wmaps[f"W{l}_{r}"] = np.ascontiguousarray(
                w.reshape(kt, P, H).transpose(1, 0, 2).astype(NPBF16)
            )

    in_maps = []
    for c in range(NC):
        m = dict(
            feat0T=f0[c],
            feat1T=f1[c],
            a00_idx=d00[c]["idx"],
            a00_roff=d00[c]["roff"],
            a00_val=d00[c]["val"],
            a01_idx=d01[c]["idx"],
            a01_roff=d01[c]["roff"],
            a01_val=d01[c]["val"],
            a10_idx=d10[c]["idx"],
            a10_roff=d10[c]["roff"],
            a10_val=d10[c]["val"],
            A11aT=da[c],
            A11bT=db[c],
            iotaC=np.ascontiguousarray(
                np.tile(np.arange(P, dtype=np.float32), (P, 1))
            ),
            identC=np.eye(P, dtype=np.float32).astype(NPBF16),
        )
        m.update(wmaps)
        in_maps.append(m)

    nc = build_program(cfg, nch00, nch01, nch10)
    prepare.last_sched = (nch00, nch01, nch10)
    return nc, in_maps


def assemble(cfg, results):
    NC = cfg["NC"]
    N0, N1 = cfg["N0"], cfg["N1"]
    full0 = np.concatenate([np.asarray(r["out0"]) for r in results], axis=0)[:N0]
    full1 = np.concatenate([np.asarray(r["out1"]) for r in results], axis=0)[:N1]
    return np.concatenate([full0, full1], axis=0).astype(np.float32)


_KERNEL_KW = {}


def kernel(
    feat0, feat1,
    a00_row, a00_col, a00_val, a01_row, a01_col, a01_val,
    a10_row, a10_col, a10_val, a11a_row, a11a_col, a11a_val,
    a11b_row, a11b_col, a11b_val,
    W1_00, W1_01, W1_10, W1_11a, W1_11b,
    W2_00, W2_01, W2_10, W2_11a, W2_11b,
    W3_00, W3_01, W3_10, W3_11a, W3_11b,
):
    inputs = dict(
        feat0=feat0, feat1=feat1,
        a00_row=a00_row, a00_col=a00_col, a00_val=a00_val,
        a01_row=a01_row, a01_col=a01_col, a01_val=a01_val,
        a10_row=a10_row, a10_col=a10_col, a10_val=a10_val,
        a11a_row=a11a_row, a11a_col=a11a_col, a11a_val=a11a_val,
        a11b_row=a11b_row, a11b_col=a11b_col, a11b_val=a11b_val,
    )
    inputs.update(
        W1_00=W1_00, W1_01=W1_01, W1_10=W1_10, W1_11a=W1_11a, W1_11b=W1_11b,
        W2_00=W2_00, W2_01=W2_01, W2_10=W2_10, W2_11a=W2_11a, W2_11b=W2_11b,
        W3_00=W3_00, W3_01=W3_01, W3_10=W3_10, W3_11a=W3_11a, W3_11b=W3_11b,
    )
    cfg = default_cfg()
    nc, in_maps = prepare(cfg, inputs)
    res = run_bass_kernel_spmd(
        nc, in_maps, list(range(cfg["NC"])), **_KERNEL_KW
    )
    kernel.last_results = res
    return assemble(cfg, res.results)
